# revision 14
# baseline (speedup 1.0000x reference)
"""Trainium2 Bass kernel for nn_DeformConvOriginalDepthWise.

Deformable depthwise conv: pointwise 1x1 conv -> offset 1x1 conv ->
bilinear gather -> depthwise 3x3 accumulation.

Sharding: 8 cores = (batch b = core//4) x (row-quarter q = core%4, 24 image
rows each). Each core redundantly computes the full xp^T of its batch (cheap
matmul), writes it to a zero-padded row-pair-interleaved DRAM layout D, then
gathers all 4 bilinear corners of each (position, kernel-point) sample with a
single dma_gather descriptor and combines on the vector engine.

Self-contained: hardcodes shapes; only needs the concourse runtime on the
import path (installed under /opt/trn_rl_repo in this environment).
"""

import sys

for _p in ("/opt/trn_rl_repo",):
    if _p not in sys.path:
        sys.path.insert(0, _p)

import numpy as np

# ---------------- problem constants (hardcoded per contract) ----------------
B = 2
C = 256  # Cin == Cout
H = W = 96
HW = H * W
K = 3
KK = K * K
PAD = 1

NCORES = 8
QUARTERS = 4
QROWS = H // QUARTERS  # 24 image rows per core
NPOS = QROWS * W  # 2304 positions per core
PT = 128  # positions per tile
NTILE = NPOS // PT  # 18

# padded gather layout: cell (yp, xq), yp in [0,99), xq in [0,100)
# holds [xp_pad[yp, xq, :] | xp_pad[yp+1, xq, :]]  (xp_pad row r = image row r-2)
DP = 100  # padded width
DROWS = 99 * DP  # 9900 cells
CELL = 2 * C  # 512 fp16 per cell
GATHER_ELEM = 2 * CELL  # 1024 fp16 = one cell pair = all 4 corners
NIDX = KK * PT  # 1152 gather indices per position tile

_cached = None


# ---------------------------------------------------------------------------
def _build(skip=(), reps=1):
    """Build + compile the SPMD bass program once. Returns (nc, meta)."""
    import concourse.bacc as bacc
    import concourse.mybir as mybir
    import concourse.tile as tile
    import dataclasses

    f32 = mybir.dt.float32
    f16 = mybir.dt.float16
    i16 = mybir.dt.int16
    i32 = mybir.dt.int32

    nc = bacc.Bacc(
        "TRN2",
        target_bir_lowering=False,
        debug=False,
        num_devices=NCORES,
    )

    # ---------------- dram parameters (per-core data) ----------------
    xb_d = nc.dram_tensor("xb", [C, HW], f32, kind="ExternalInput")
    xown_d = nc.dram_tensor("xown", [C, NPOS], f32, kind="ExternalInput")
    pwT_d = nc.dram_tensor("pwT", [C, C], f32, kind="ExternalInput")  # pw_w.T
    woffT_d = nc.dram_tensor("woffT", [C, 2 * KK], f32, kind="ExternalInput")
    bias18_d = nc.dram_tensor("bias18", [128, 2 * KK], f32, kind="ExternalInput")
    base_d = nc.dram_tensor("base", [128, NTILE * 2 * KK], f32, kind="ExternalInput")
    dwb_d = nc.dram_tensor("dwb", [128, KK * C], f16, kind="ExternalInput")
    out_d = nc.dram_tensor("out", [NPOS, C], f32, kind="ExternalOutput")

    # scratch in HBM
    D_d = nc.dram_tensor("Dsc", [DROWS, CELL], f16)
    idxb_d = nc.dram_tensor("idxb", [128, NTILE * KK], i16)

    with tile.TileContext(nc) as tc:
      for _rep in range(reps):
        with (
            tc.tile_pool(name="const", bufs=1) as cpool,
            tc.tile_pool(name="xin", bufs=3) as xpool,
            tc.tile_pool(name="xprow", bufs=4) as rpool,
            tc.tile_pool(name="coord", bufs=1) as opool,
            tc.tile_pool(name="gather", bufs=3) as gpool,
            tc.tile_pool(name="acc", bufs=3) as apool,
            tc.tile_pool(name="outp", bufs=3) as outpool,
            tc.tile_pool(name="psum", bufs=4, space="PSUM") as psum,
            tc.tile_pool(name="psum_off", bufs=4, space="PSUM") as psum_off,
        ):
            # ---------------- constants ----------------
            pwT = cpool.tile([128, 2, C], f32)
            nc.sync.dma_start(
                pwT[:], pwT_d[:].rearrange("(k p) n -> p k n", k=2)
            )
            woffT = cpool.tile([128, 2, 2 * KK], f32)
            nc.sync.dma_start(
                woffT[:], woffT_d[:].rearrange("(k p) n -> p k n", k=2)
            )
            bias18 = cpool.tile([128, 2 * KK], f32)
            nc.sync.dma_start(bias18[:], bias18_d[:])
            basec = cpool.tile([128, NTILE * 2 * KK], f32)
            nc.sync.dma_start(basec[:], base_d[:])
            dwb = cpool.tile([128, KK * C], f16)
            nc.sync.dma_start(dwb[:], dwb_d[:])
            xown = cpool.tile([128, 2, NPOS], f32)
            nc.sync.dma_start(
                xown[:], xown_d[:].rearrange("(k p) n -> p k n", k=2)
            )
            zero16 = cpool.tile([128, 1024], f16)
            nc.vector.memset(zero16[:], 0.0)

            # ---------------- zero D pad regions ----------------
            Dflat = D_d[:].rearrange("r c -> (r c)")
            # rows 0..1 (200 cells) and rows 97..98 (200 cells)
            for cell0 in (0, 97 * DP) if "dzero" not in skip else ():
                nc.sync.dma_start(
                    Dflat[cell0 * CELL : (cell0 + 200) * CELL].rearrange(
                        "(p n) -> p n", p=128
                    ),
                    zero16[:, :800],
                )
            # columns 0..1 and 98..99 of rows 2..96 (95 rows x 2 cells each)
            # 95*2*512 = 97280 fp16 = 128 x 760
            for col0 in (0, 98) if "dzero" not in skip else ():
                dst = dataclasses.replace(
                    Dflat,
                    offset=Dflat.offset + (2 * DP + col0) * CELL,
                    ap=[[DP * CELL, 95], [1, 2 * CELL]],
                )
                nc.sync.dma_start(dst, zero16[:95, :])

            # ---------------- offsets for own quarter ----------------
            offsb = opool.tile([128, NTILE * 2 * KK], f32, tag="offsb")
            for t in range(NTILE):
                po = psum_off.tile([128, 2 * KK], f32)
                for k in range(2):
                    nc.tensor.matmul(
                        po[:],
                        xown[:, k, t * PT : (t + 1) * PT],
                        woffT[:, k, :],
                        start=(k == 0),
                        stop=(k == 1),
                    )
                nc.vector.tensor_add(
                    offsb[:, t * 2 * KK : (t + 1) * 2 * KK], po[:], bias18[:]
                )

            # ---------------- coords -> floor, frac, weights, idx ----------
            NF = NTILE * 2 * KK  # 648
            coords = opool.tile([128, NF], f32, tag="coords")
            nc.vector.tensor_add(coords[:], offsb[:], basec[:])
            ci32 = opool.tile([128, NF], i32, tag="ci32")
            nc.vector.tensor_copy(ci32[:], coords[:])
            tback = opool.tile([128, NF], f32, tag="tback")
            nc.vector.tensor_copy(tback[:], ci32[:])
            gt = opool.tile([128, NF], f32, tag="gt")
            nc.vector.tensor_tensor(gt[:], tback[:], coords[:], mybir.AluOpType.is_gt)
            fl = opool.tile([128, NF], f32, tag="fl")
            nc.vector.tensor_tensor(fl[:], tback[:], gt[:], mybir.AluOpType.subtract)
            frac = opool.tile([128, NF], f32, tag="frac")
            nc.vector.tensor_tensor(frac[:], coords[:], fl[:], mybir.AluOpType.subtract)
            om = opool.tile([128, NF], f32, tag="om")
            nc.vector.tensor_scalar(
                om[:], frac[:], -1.0, 1.0, mybir.AluOpType.mult, mybir.AluOpType.add
            )
            flc = opool.tile([128, NF], f32, tag="flc")
            nc.vector.tensor_scalar(
                flc[:], fl[:], -2.0, 96.0, mybir.AluOpType.max, mybir.AluOpType.min
            )
            # idx = 100*fy + fx + 202   over [128, NTILE*KK]
            flc_v = flc[:].rearrange("p (t i two) -> p t i two", t=NTILE, two=2)
            idxf = opool.tile([128, NTILE * KK], f32, tag="idxf")
            idxf_v = idxf[:].rearrange("p (t i) -> p t i", t=NTILE)
            nc.vector.scalar_tensor_tensor(
                idxf_v,
                flc_v[:, :, :, 0],
                100.0,
                flc_v[:, :, :, 1],
                mybir.AluOpType.mult,
                mybir.AluOpType.add,
            )
            idxf2 = opool.tile([128, NTILE * KK], f32, tag="idxf2")
            nc.vector.tensor_scalar_add(idxf2[:], idxf[:], 202.0)
            idx16 = opool.tile([128, NTILE * KK], i16, tag="idx16")
            nc.vector.tensor_copy(idx16[:], idxf2[:])

            # weights [128, (t, kk, 4)] f32; chunk order in gather elem:
            # 0:(y0,x0)=omy*omx 1:(y1,x0)=fy*omx 2:(y0,x1)=omy*fx 3:(y1,x1)=fy*fx
            wts = opool.tile([128, NTILE * KK * 4], f32, tag="wts")
            wts_v = wts[:].rearrange("p (t i c) -> p t i c", t=NTILE, i=KK)
            om_v = om[:].rearrange("p (t i two) -> p t i two", t=NTILE, two=2)
            fr_v = frac[:].rearrange("p (t i two) -> p t i two", t=NTILE, two=2)
            for ci, (ya, xa) in enumerate(((om_v, om_v), (fr_v, om_v), (om_v, fr_v), (fr_v, fr_v))):
                nc.vector.tensor_tensor(
                    wts_v[:, :, :, ci],
                    ya[:, :, :, 0],
                    xa[:, :, :, 1],
                    mybir.AluOpType.mult,
                )

            # ---------------- idx shuffle to wrapped layout ----------------
            wrap = cpool.tile([128, NTILE * KK * 8], i16)
            if "shuffle" in skip:
                nc.vector.memset(wrap[:], 0)
            else:
                # store idx16 [p=(q,r), (t,i)] -> DRAM [r:16, t, i, q:8] (strided)
                idst = dataclasses.replace(
                    idxb_d[:].rearrange("a b -> (a b)"),
                    ap=[[1, 8], [NTILE * KK * 8, 16], [KK * 8, NTILE], [8, KK]],
                )
                nc.sync.dma_start(idst, idx16[:])
                # load wrapped [16, 1296] then replicate by partition doubling
                wsrc = idxb_d[:].rearrange("a b -> (a b)")[: 16 * NTILE * KK * 8]
                nc.sync.dma_start(
                    wrap[0:16, :], wsrc.rearrange("(p n) -> p n", p=16)
                )
                for g in (16, 32, 64):
                    nc.sync.dma_start(wrap[g : 2 * g, :], wrap[0:g, :])

            # ---------------- xp matmul + D stores ----------------
            ROWCHUNK = 8  # image rows per x DMA
            for ch in range(H // ROWCHUNK):
                xch = xpool.tile([128, 2, ROWCHUNK * W], f32)
                nc.sync.dma_start(
                    xch[:],
                    xb_d[:].rearrange("(k p) n -> p k n", k=2)[
                        :, :, ch * ROWCHUNK * W : (ch + 1) * ROWCHUNK * W
                    ],
                )
                xp8 = rpool.tile([W, ROWCHUNK, C], f16)
                for y8 in range(ROWCHUNK):
                    pxp = psum.tile([W, C], f32)
                    for k in range(2):
                        nc.tensor.matmul(
                            pxp[:],
                            xch[:, k, y8 * W : (y8 + 1) * W],
                            pwT[:, k, :],
                            start=(k == 0),
                            stop=(k == 1),
                        )
                    nc.scalar.copy(xp8[:, y8, :], pxp[:])
                # batched store of 8 rows: half1 cells (y+2, 2..98),
                # half2 cells (y+1, 2..98); src iterates (w-part, y8, c)
                y0 = ch * ROWCHUNK
                for half, yp0 in ((0, y0 + 2), (1, y0 + 1)) if "dstore" not in skip else ():
                    dst = dataclasses.replace(
                        Dflat,
                        offset=Dflat.offset + (yp0 * DP + 2) * CELL + half * C,
                        ap=[[CELL, W], [DP * CELL, ROWCHUNK], [1, C]],
                    )
                    nc.sync.dma_start(dst, xp8[:])

            # ---------------- gather + combine per tile ----------------
            Dsrc = dataclasses.replace(
                Dflat, ap=[[CELL, DROWS - 1], [1, GATHER_ELEM]]
            )
            for t in range(NTILE):
                G = gpool.tile([128, KK, GATHER_ELEM], f16)
                if "gather" in skip:
                    nc.vector.memset(G[:], 0.0)
                else:
                    # SWDGE ring holds 1024 descriptors -> split 9*128 idxs
                    for i0, i1 in ((0, 5), (5, KK)):
                        n = (i1 - i0) * PT
                        nc.gpsimd.dma_gather(
                            G[:, i0:i1, :],
                            Dsrc,
                            wrap[:, t * KK * 8 + i0 * 8 : t * KK * 8 + i1 * 8],
                            n,
                            n,
                            GATHER_ELEM,
                            elem_step=CELL,
                        )
                acc = apool.tile([128, KK, C], f16)
                for kk in range(KK):
                    wcol = lambda ci: wts[:, t * KK * 4 + kk * 4 + ci : t * KK * 4 + kk * 4 + ci + 1]
                    nc.vector.tensor_scalar_mul(
                        acc[:, kk, :], G[:, kk, 0:C], wcol(0)
                    )
                    for ci in range(1, 4):
                        nc.vector.scalar_tensor_tensor(
                            acc[:, kk, :],
                            G[:, kk, ci * C : (ci + 1) * C],
                            wcol(ci),
                            acc[:, kk, :],
                            mybir.AluOpType.mult,
                            mybir.AluOpType.add,
                        )
                dwm = apool.tile([128, KK, C], f16, tag="dwm")
                nc.vector.tensor_tensor(
                    dwm[:].rearrange("p a b -> p (a b)"),
                    acc[:].rearrange("p a b -> p (a b)"),
                    dwb[:],
                    mybir.AluOpType.mult,
                )
                ot = outpool.tile([128, C], f32)
                nc.vector.tensor_reduce(
                    ot[:],
                    dwm[:].transpose([0, 2, 1]),
                    mybir.AxisListType.X,
                    mybir.AluOpType.add,
                )
                nc.sync.dma_start(out_d[t * PT : (t + 1) * PT, :], ot[:])

    nc.compile()
    return nc


def _host_inputs(inputs):
    """Build the 8 per-core input maps from the full problem inputs."""
    x = np.ascontiguousarray(np.asarray(inputs["x"], dtype=np.float32))
    pw_w = np.asarray(inputs["pw_w"], dtype=np.float32)
    off_w = np.asarray(inputs["off_w"], dtype=np.float32)
    off_b = np.asarray(inputs["off_b"], dtype=np.float32)
    dw_w = np.asarray(inputs["dw_w"], dtype=np.float32)

    woff = (off_w @ pw_w).astype(np.float32)  # [18, 256] folded offset conv
    pwT = np.ascontiguousarray(pw_w.T)
    woffT = np.ascontiguousarray(woff.T)  # [256, 18]
    bias18 = np.tile(off_b[None, :], (128, 1)).astype(np.float32)
    dwb = np.tile(
        np.ascontiguousarray(dw_w.reshape(C, KK).T).reshape(1, KK * C), (128, 1)
    ).astype(np.float16)

    ky, kx = np.meshgrid(np.arange(K), np.arange(K), indexing="ij")
    ky = ky.ravel().astype(np.float32)
    kx = kx.ravel().astype(np.float32)

    in_maps = []
    for core in range(NCORES):
        b = core // QUARTERS
        q = core % QUARTERS
        h0 = q * QROWS
        pos = np.arange(h0 * W, (h0 + QROWS) * W)
        hh = (pos // W).astype(np.float32)
        ww = (pos % W).astype(np.float32)
        # base [pos, (kk, 2)] interleaved y,x
        basef = np.empty((NPOS, 2 * KK), np.float32)
        basef[:, 0::2] = hh[:, None] - 1.0 + ky[None, :]
        basef[:, 1::2] = ww[:, None] - 1.0 + kx[None, :]
        # tile layout [128, (t, c)]
        base_t = np.ascontiguousarray(
            basef.reshape(NTILE, PT, 2 * KK).transpose(1, 0, 2).reshape(PT, -1)
        )
        xb = np.ascontiguousarray(x[b].reshape(C, HW))
        xown = np.ascontiguousarray(xb[:, pos])
        in_maps.append(
            {
                "xb": xb,
                "xown": xown,
                "pwT": pwT,
                "woffT": woffT,
                "bias18": bias18,
                "base": base_t,
                "dwb": dwb,
            }
        )
    return in_maps


def kernel(**inputs) -> np.ndarray:
    global _cached
    from concourse.bass_utils import run_bass_kernel_spmd

    if _cached is None:
        _cached = _build()
    nc = _cached

    in_maps = _host_inputs(inputs)
    res = run_bass_kernel_spmd(nc, in_maps, list(range(NCORES)))
    out = np.zeros((B, C, H, W), np.float32)
    for core in range(NCORES):
        b = core // QUARTERS
        q = core % QUARTERS
        o = res.results[core]["out"]  # [NPOS, C]
        out[b, :, q * QROWS : (q + 1) * QROWS, :] = (
            o.reshape(QROWS, W, C).transpose(2, 0, 1)
        )
    return out


if __name__ == "__main__":
    # smoke: build only
    _build()
    print("build ok")
